# revision 1
# baseline (speedup 1.0000x reference)
import sys
import numpy as np
from contextlib import ExitStack

sys.path.insert(0, "/opt/trn_rl_repo")

import concourse.bass as bass
import concourse.bacc as bacc
import concourse.mybir as mybir
import concourse.tile as tile
from concourse.bass_utils import run_bass_kernel_spmd
from concourse.masks import make_identity

f32 = mybir.dt.float32
u32 = mybir.dt.uint32
Copy = mybir.ActivationFunctionType.Copy
Ident = mybir.ActivationFunctionType.Identity
EQ = mybir.AluOpType.is_equal
SUB = mybir.AluOpType.subtract
NEG = -1.0e30
SENT = 2.0

N = 4096
QH = 2048
QB = 16
K = 20

_NC_CACHE = []


def _build():
    nc = bacc.Bacc("TRN2", target_bir_lowering=False, debug=False, num_devices=8)

    xT_d = nc.dram_tensor("xT", (3, N), f32, kind="ExternalInput")
    xTq_d = nc.dram_tensor("xTq", (3, QH), f32, kind="ExternalInput")
    sqrow_d = nc.dram_tensor("sqrow", (1, N), f32, kind="ExternalInput")
    sq2dq_d = nc.dram_tensor("sq2dq", (128, QB), f32, kind="ExternalInput")
    w1t_d = nc.dram_tensor("w1t", (3, 64), f32, kind="ExternalInput")
    w2t_d = nc.dram_tensor("w2t", (64, 64), f32, kind="ExternalInput")
    w3t_d = nc.dram_tensor("w3t", (64, 64), f32, kind="ExternalInput")
    w4t_d = nc.dram_tensor("w4t", (64, 128), f32, kind="ExternalInput")
    bpre_d = nc.dram_tensor("bpre", (128, 4), f32, kind="ExternalInput")
    wl_d = [
        nc.dram_tensor("wl0", (128, 2560), f32, kind="ExternalInput"),
        nc.dram_tensor("wl1", (128, 2560), f32, kind="ExternalInput"),
        nc.dram_tensor("wl2", (128, 2560), f32, kind="ExternalInput"),
        nc.dram_tensor("wl3", (128, 5120), f32, kind="ExternalInput"),
    ]
    blpost_d = nc.dram_tensor("blpost", (128, 8), f32, kind="ExternalInput")
    out_d = nc.dram_tensor("out", (1024, QH), f32, kind="ExternalOutput")
    Fall_d = nc.dram_tensor("Fall", (N, 320), f32, kind="Internal")

    with ExitStack() as ctx:
        tc = ctx.enter_context(tile.TileContext(nc))
        const = ctx.enter_context(tc.tile_pool(name="const", bufs=1))
        psum = ctx.enter_context(tc.tile_pool(name="psum", bufs=2, space="PSUM"))

        def load(shape, dt, dram, tag):
            t = const.tile(list(shape), dt, tag=tag)
            nc.sync.dma_start(t[:], dram[:])
            return t

        xT_s = load((3, N), f32, xT_d, "xT")
        sq2dq_s = load((128, QB), f32, sq2dq_d, "sq2dq")
        w1t_s = load((3, 64), f32, w1t_d, "w1t")
        w2t_s = load((64, 64), f32, w2t_d, "w2t")
        w3t_s = load((64, 64), f32, w3t_d, "w3t")
        w4t_s = load((64, 128), f32, w4t_d, "w4t")
        bpre_s = load((128, 4), f32, bpre_d, "bpre")
        blpost_s = load((128, 8), f32, blpost_d, "blpost")
        wl_s = [
            load((128, 2560), f32, wl_d[0], "wl0"),
            load((128, 2560), f32, wl_d[1], "wl1"),
            load((128, 2560), f32, wl_d[2], "wl2"),
            load((128, 5120), f32, wl_d[3], "wl3"),
        ]

        ident = const.tile([128, 128], f32, tag="id")
        make_identity(nc, ident[:])
        ones = const.tile([1, 128], f32, tag="ones")
        nc.vector.memset(ones[:], 1.0)

        # PE fences: one tiny matmul per PE-read tensor so hot-loop matmuls
        # carry at most one semaphore wait
        fps = psum.tile([1, 1], f32, tag="fence", bufs=1)
        for ft in (ones, xT_s, w1t_s, w2t_s, w3t_s, w4t_s,
                   wl_s[0], wl_s[1], wl_s[2], wl_s[3], ident):
            nc.tensor.matmul(fps[:], ft[0:1, 0:1], ft[0:1, 0:1])

        sqm_b = const.tile([128, N], f32, tag="sqm")
        with tc.tile_pool(name="init", bufs=1) as initp:
            sqrow_s = initp.tile([1, N], f32, tag="sqrow")
            nc.sync.dma_start(sqrow_s[:], sqrow_d[:])
            nc.tensor.matmul(fps[:], sqrow_s[0:1, 0:1], sqrow_s[0:1, 0:1])
            for j in range(8):
                ps = psum.tile([128, 512], f32, tag="pse")
                nc.tensor.matmul(ps[:], ones[:], sqrow_s[:, j * 512:(j + 1) * 512])
                nc.scalar.activation(sqm_b[:, j * 512:(j + 1) * 512], ps[:], Copy)

        # Phase B: xc chain + packed gather table Fall (row n = all 320 features)
        with tc.tile_pool(name="pb", bufs=1) as pb:
            cur = xT_s
            stages = [(w1t_s, 64, 0), (w2t_s, 64, 64), (w3t_s, 64, 128),
                      (w4t_s, 128, 192)]
            for s, (wt, Cout, soff) in enumerate(stages):
                xc = pb.tile([Cout, N], f32, tag=f"xc{s % 2}")
                for j in range(8):
                    ps = psum.tile([128, 512], f32, tag="pse")
                    nc.tensor.matmul(ps[0:Cout, :], wt[:], cur[:, j * 512:(j + 1) * 512])
                    nc.scalar.activation(xc[:, j * 512:(j + 1) * 512], ps[0:Cout, :],
                                         Ident, bias=bpre_s[0:Cout, s:s + 1])
                per = 512 // Cout
                for grp in range(32 // per):
                    pst = psum.tile([128, 512], f32, tag="pstr")
                    for u in range(per):
                        g = grp * per + u
                        nc.tensor.transpose(pst[:, u * Cout:(u + 1) * Cout],
                                            xc[:, g * 128:(g + 1) * 128],
                                            ident[0:Cout, 0:Cout])
                    fst = pb.tile([128, 512], f32, tag="fst", bufs=2)
                    nc.scalar.activation(fst[:], pst[:], Copy)
                    for u in range(per):
                        g = grp * per + u
                        nc.gpsimd.dma_start(
                            Fall_d[g * 128:(g + 1) * 128, soff:soff + Cout],
                            fst[:, u * Cout:(u + 1) * Cout])
                cur = xc

        # Phase A (knn topk per 128-query block) interleaved with Phase C
        pa = ctx.enter_context(tc.tile_pool(name="pa", bufs=1))
        pc = ctx.enter_context(tc.tile_pool(name="pc", bufs=1))
        idx_tiles = {}

        def emit_A(t):
            lhsA = pa.tile([3, 128], f32, tag="lhsA", bufs=2)
            nc.sync.dma_start(lhsA[:], xTq_d[:, t * 128:(t + 1) * 128])
            nc.tensor.matmul(fps[:], lhsA[0:1, 0:1], lhsA[0:1, 0:1])
            e2 = pa.tile([128, N], f32, tag="e2")
            for mb in range(8):
                ps = psum.tile([128, 512], f32, tag="pse")
                nc.tensor.matmul(ps[:], lhsA[:],
                                 xT_s[:, mb * 512:(mb + 1) * 512])
                nc.scalar.activation(e2[:, mb * 512:(mb + 1) * 512], ps[:], Copy,
                                     scale=2.0)
            sT = pa.tile([128, N], f32, tag="s_")
            nc.scalar.activation(sT[:], sqm_b[:], Ident, bias=sq2dq_s[:, t:t + 1])
            t_ = pa.tile([128, N], f32, tag="Atmp")
            nc.vector.tensor_sub(t_[:], e2[:], sT[:])
            Aw = pa.tile([128, N], f32, tag="e2")
            nc.scalar.activation(Aw[:], t_[:], Copy, bias=-1e-7)
            idx_t = pa.tile([128, 24], u32, tag="idx", bufs=6)
            idx_tiles[t] = idx_t

            # top-24 in 3 rounds of sorted max8; max_index/match_replace both
            # claim successive occurrences for duplicate needles, which matches
            # jax top_k ascending-index tie order (verified on device)
            A_in = Aw
            for r in range(3):
                m = pa.tile([128, 8], f32, tag="m", bufs=2)
                nc.vector.max(m[:], A_in[:])
                nc.vector.max_index(idx_t[:, r * 8:(r + 1) * 8], m[:], A_in[:])
                if r < 2:
                    A_nxt = pa.tile([128, N], f32,
                                    tag=("s_" if r == 0 else "Atmp"))
                    nc.vector.match_replace(A_nxt[:], m[:], A_in[:], NEG)
                    A_in = A_nxt

        def emit_C(t):
            idx_t = idx_tiles[t]
            # G[p, k*320 + c] = Fall[idx[p,k], c]; per-row layout
            # [s0 c<64 | s1 c<64 | s2 c<64 | s3 c<128]
            G = pc.tile([128, 6400], f32, tag="G")
            for k in range(K):
                nc.gpsimd.indirect_dma_start(
                    out=G[:, k * 320:(k + 1) * 320], out_offset=None,
                    in_=Fall_d[:],
                    in_offset=bass.IndirectOffsetOnAxis(ap=idx_t[:, k:k + 1],
                                                        axis=0))
            nc.tensor.matmul(fps[:], G[0:1, 6399:6400], G[0:1, 6399:6400])
            for s in range(4):
                nslab = 10 if s < 3 else 20
                GT = pc.tile([128, nslab * 128], f32, tag="GT")
                if s < 3:
                    Gs = pc.tile([128, 1280], f32, tag="Gs")
                    for k in range(K):
                        nc.scalar.activation(
                            Gs[:, k * 64:(k + 1) * 64],
                            G[:, k * 320 + s * 64:k * 320 + (s + 1) * 64], Copy)
                    nc.tensor.matmul(fps[:], Gs[0:1, 1279:1280],
                                     Gs[0:1, 1279:1280])
                for grp in range((nslab + 3) // 4):
                    un = min(4, nslab - grp * 4)
                    pst = psum.tile([128, 512], f32, tag="pstr")
                    for u in range(un):
                        j = grp * 4 + u
                        if s < 3:
                            src = Gs[:, j * 128:(j + 1) * 128]
                        else:
                            src = G[:, j * 320 + 192:j * 320 + 320]
                        nc.tensor.transpose(pst[:, u * 128:(u + 1) * 128],
                                            src, ident[:])
                    nc.scalar.activation(GT[:, grp * 512:grp * 512 + un * 128],
                                         pst[:, 0:un * 128], Copy)
                nc.tensor.matmul(fps[:], GT[0:1, nslab * 128 - 1:nslab * 128],
                                 GT[0:1, nslab * 128 - 1:nslab * 128])
                wl = wl_s[s]
                for oh in range(2):
                    pco = psum.tile([128, 128], f32, tag="psc")
                    for j in range(nslab):
                        nc.tensor.matmul(pco[:],
                                         wl[:, j * 256 + oh * 128:j * 256 + (oh + 1) * 128],
                                         GT[:, j * 128:(j + 1) * 128],
                                         start=(j == 0), stop=(j == nslab - 1))
                    ob = pc.tile([128, 128], f32, tag="ob", bufs=2)
                    nc.scalar.activation(ob[:], pco[:], Ident,
                                         bias=blpost_s[:, s * 2 + oh:s * 2 + oh + 1])
                    nc.sync.dma_start(
                        out_d[s * 256 + oh * 128:s * 256 + (oh + 1) * 128,
                              t * 128:(t + 1) * 128], ob[:])

        emit_A(0)
        for t in range(1, QB):
            emit_A(t)
            emit_C(t - 1)
        emit_C(QB - 1)

    nc.compile()
    return nc


def kernel(**inputs):
    x = np.asarray(inputs["x"], dtype=np.float32)
    W = {k: np.asarray(inputs[k], dtype=np.float32)
         for k in inputs if k != "x"}
    B = x.shape[0]

    if not _NC_CACHE:
        _NC_CACHE.append(_build())
    nc = _NC_CACHE[0]

    bpre = np.zeros((128, 4), np.float32)
    bpre[0:64, 0] = W["b1"]
    bpre[0:64, 1] = W["b2"]
    bpre[0:64, 2] = W["b3"]
    bpre[0:128, 3] = W["b4"]
    blpost = np.zeros((128, 8), np.float32)
    for s, nm in enumerate(["bL2", "bL3", "bL4", "bL5"]):
        for oh in range(2):
            blpost[:, s * 2 + oh] = W[nm][oh * 128:(oh + 1) * 128]
    wl = [
        np.ascontiguousarray(W["WL2"].reshape(256, 10, 2, 64).transpose(2, 3, 1, 0).reshape(128, 2560)),
        np.ascontiguousarray(W["WL3"].reshape(256, 10, 2, 64).transpose(2, 3, 1, 0).reshape(128, 2560)),
        np.ascontiguousarray(W["WL4"].reshape(256, 10, 2, 64).transpose(2, 3, 1, 0).reshape(128, 2560)),
        np.ascontiguousarray(W["WL5"].reshape(256, 20, 128).transpose(2, 1, 0).reshape(128, 5120)),
    ]
    w1t = np.ascontiguousarray(W["W1"].T)
    w2t = np.ascontiguousarray(W["W2"].T)
    w3t = np.ascontiguousarray(W["W3"].T)
    w4t = np.ascontiguousarray(W["W4"].T)

    in_maps = []
    for c in range(8):
        b, h = c // 2, c % 2
        xT = np.ascontiguousarray(x[b].T)
        sq = (x[b].astype(np.float32) ** 2).sum(axis=-1, dtype=np.float32)
        in_maps.append({
            "xT": xT,
            "xTq": np.ascontiguousarray(xT[:, h * QH:(h + 1) * QH]),
            "sqrow": sq.reshape(1, N).copy(),
            "sq2dq": np.ascontiguousarray(sq[h * QH:(h + 1) * QH].reshape(QB, 128).T),
            "w1t": w1t, "w2t": w2t, "w3t": w3t, "w4t": w4t,
            "bpre": bpre, "blpost": blpost,
            "wl0": wl[0], "wl1": wl[1], "wl2": wl[2], "wl3": wl[3],
        })

    res = run_bass_kernel_spmd(nc, in_maps, core_ids=list(range(8)))
    full = np.empty((B, 1024, N), np.float32)
    for c in range(8):
        b, h = c // 2, c % 2
        full[b][:, h * QH:(h + 1) * QH] = res.results[c]["out"]
    return full

